# revision 1
# baseline (speedup 1.0000x reference)
"""CondConv (per-sample dynamic conv) Trainium2 Bass kernel.

Reference computation (per sample b):
    gap     = mean(x[b], spatial)                    # [C]
    r       = sigmoid(fc_w @ gap + fc_b)             # [E]
    comb    = sum_e r[e] * kernel_weights[e]         # [O, I, 3, 3]
    y[b]    = conv2d(x[b], comb, pad=1)              # [O, H, W]

Sharding: data-parallel over batch, 4 samples per core on 8 cores.
Expert kernels + fc params replicated to every core.

Per-core dataflow:
  - x[s] DMA'd (contiguous, fp32) into a staging tile
  - one ScalarE activation(Copy) pass per ci-half casts staging ->
    bf16 zero-padded conv layout [128p=i%128, ci, 58, 58] AND
    accumulates the spatial sum (accum_out) for GAP in fp32
  - routing: PE matmul (gap x fc_wT/HW) -> +bias (DVE) -> sigmoid (ACT)
    -> broadcast to 128 partitions via K=1 ones-matmul (PE)
  - synthesis on VectorE: 8 fused scalar_tensor_tensor passes over the
    bf16 expert stack -> bf16 combined weights, laid out so conv lhsT
    slices are contiguous [i, oh, ci, tap, oin]
  - conv: per (oh, ntile) PSUM tile [128, 448] accumulates 18 bf16
    matmuls (2 ci halves x 9 taps), fp32 in PSUM
  - ScalarE copies PSUM->SBUF fp32, HWDGE(ACT ring) DMAs to HBM

The program is software-pipelined so staging/routing/synthesis of
sample s+1 run (DMA/DVE/ACT) underneath the conv matmuls of sample s
(PE).
"""

import numpy as np
import ml_dtypes

B, C, H, W = 32, 256, 56, 56
E = 8
N_CORES = 8
BL = B // N_CORES          # local batch per core
HP = 58                    # padded rows (1 top + 1 bottom)
WP = 60                    # padded cols (2 left + 2 right: keeps the
                           # bf16 interior 4B-aligned for DVE/ACT 2x)
HWP = HP * WP              # 3480
HWU = H * W                # 3136 (unpadded)
TAPS = 9
OIN = 128                  # output channels per half
EBLK = 2 * 2 * TAPS * OIN  # per-partition free elems per expert = 4608
OHBLK = 2 * TAPS * OIN     # per (oh) block = 2304
CIBLK = TAPS * OIN         # per (oh, ci) block = 1152
ROWS = 8                   # output rows per n-tile
NT = H // ROWS             # 7 n-tiles
NF = ROWS * W              # 448 matmul free dim

_CACHE = {}


def _build():
    import concourse.bacc as bacc
    import concourse.mybir as mybir
    import concourse.tile as tile
    from contextlib import ExitStack

    dt = mybir.dt
    AF = mybir.ActivationFunctionType
    Alu = mybir.AluOpType

    nc = bacc.Bacc(
        "TRN2",
        target_bir_lowering=False,
        debug=False,
        enable_asserts=False,
        num_devices=N_CORES,
    )
    x_d = nc.dram_tensor("x", [BL, C, H, W], dt.float32, kind="ExternalInput")
    w_d = nc.dram_tensor("wp", [128, E * EBLK], dt.bfloat16, kind="ExternalInput")
    fcw_d = nc.dram_tensor("fcw", [C, E], dt.float32, kind="ExternalInput")
    fcb_d = nc.dram_tensor("fcb", [E, 1], dt.float32, kind="ExternalInput")
    eye_d = nc.dram_tensor("eye", [E, E], dt.float32, kind="ExternalInput")
    y_d = nc.dram_tensor("y", [BL, C, H, W], dt.float32, kind="ExternalOutput")

    with tile.TileContext(nc) as tc:
        with ExitStack() as ctx:
            cpool = ctx.enter_context(tc.tile_pool(name="consts", bufs=1))
            stgpool = ctx.enter_context(tc.tile_pool(name="stg", bufs=2))
            xpool = ctx.enter_context(tc.tile_pool(name="xs", bufs=4))
            combpool = ctx.enter_context(tc.tile_pool(name="combs", bufs=2))
            spool = ctx.enter_context(tc.tile_pool(name="small", bufs=2))
            opool = ctx.enter_context(tc.tile_pool(name="outs", bufs=2))
            pspool = ctx.enter_context(tc.tile_pool(name="cpsum", bufs=5, space="PSUM"))
            psmall = ctx.enter_context(tc.tile_pool(name="spsum", bufs=1, space="PSUM"))

            w_sb = cpool.tile([128, E * EBLK], dt.bfloat16)
            fcw_sb = cpool.tile([128, 2 * E], dt.float32)
            fcb_sb = cpool.tile([E, 1], dt.float32)
            eye_sb = cpool.tile([E, E], dt.float32)

            xvs, gaps, rbs, combs = {}, {}, {}, {}

            def load_consts():
                for ci in range(2):
                    nc.sync.dma_start(
                        out=fcw_sb[:, ci * E : (ci + 1) * E],
                        in_=fcw_d.ap()[ci * 128 : (ci + 1) * 128, :],
                    )
                nc.sync.dma_start(out=fcb_sb[:], in_=fcb_d.ap())
                nc.sync.dma_start(out=eye_sb[:], in_=eye_d.ap())

            def load_w():
                # oh=0 blocks for all experts first, so sample 0's first
                # synthesis half never waits on the tail of the W load.
                # SWDGE (gpsimd) ring: keeps the big W transfer off the
                # ACT/SP HWDGE rings so it never head-of-line-blocks the
                # GAP casts or x staging DMAs.
                for oh in range(2):
                    for e in range(E):
                        lo = e * EBLK + oh * OHBLK
                        nc.gpsimd.dma_start(
                            out=w_sb[:, lo : lo + OHBLK],
                            in_=w_d.ap()[:, lo : lo + OHBLK],
                        )

            def stage(s, first=False):
                xg = stgpool.tile([128, 2 * HWU], dt.float32, tag="xg")
                xgv = xg.rearrange("p (c h w) -> p c h w", c=2, h=H, w=W)
                for ci in range(2):
                    eng = nc.scalar if (first and ci == 1) else nc.sync
                    eng.dma_start(
                        out=xgv[:, ci],
                        in_=x_d.ap()[s, ci * 128 : (ci + 1) * 128, :, :],
                    )
                xt = xpool.tile([128, 2 * HWP], dt.bfloat16, tag="xt")
                xv = xt.rearrange("p (c h w) -> p c h w", c=2, h=HP, w=WP)
                xvs[s] = xv
                for ci in range(2):
                    nc.vector.memset(xv[:, ci, 0, :], 0.0)
                    nc.vector.memset(xv[:, ci, HP - 1, :], 0.0)
                    nc.vector.memset(xv[:, ci, 1 : HP - 1, 0:2], 0.0)
                    nc.vector.memset(xv[:, ci, 1 : HP - 1, WP - 2 : WP], 0.0)
                g = spool.tile([128, 2], dt.float32, tag="gap")
                gaps[s] = g
                # cast fp32 -> bf16 into the padded layout AND reduce for GAP
                for ci in range(2):
                    nc.scalar.activation(
                        out=xv[:, ci, 1 : 1 + H, 2 : 2 + W],
                        in_=xgv[:, ci],
                        func=AF.Copy,
                        accum_out=g[:, ci : ci + 1],
                    )

            def route(s):
                # logits laid out [E partitions, 1] so fc_b rides the ACT
                # sigmoid as a per-partition bias; the whole chain is
                # PE -> ACT -> PE -> ACT (no DVE, so synthesis blobs on
                # DVE can never starve the next sample's routing).
                g = gaps[s]
                pl = psmall.tile([E, 1], dt.float32, tag="pl")
                for ci in range(2):
                    nc.tensor.matmul(
                        pl[:],
                        lhsT=fcw_sb[:, ci * E : (ci + 1) * E],
                        rhs=g[:, ci : ci + 1],
                        start=(ci == 0),
                        stop=(ci == 1),
                    )
                rr = spool.tile([E, 1], dt.float32, tag="rr")
                nc.scalar.activation(
                    out=rr[:], in_=pl[:], func=AF.Sigmoid, bias=fcb_sb[:], scale=1.0
                )
                # broadcast r to all 128 partitions: eye-matmul with a
                # stride-0 (free dim) lhsT view of the [E,1] column
                prb = psmall.tile([128, E], dt.float32, tag="prb")
                nc.tensor.matmul(
                    prb[:],
                    lhsT=rr[:].broadcast_to([E, 128]),
                    rhs=eye_sb[:],
                    start=True,
                    stop=True,
                )
                rb = spool.tile([128, E], dt.float32, tag="rb")
                nc.scalar.activation(out=rb[:], in_=prb[:], func=AF.Copy)
                rbs[s] = rb

            def synth(s):
                # scalar_tensor_tensor only has a 1x uop (fp32 scalar
                # operand), so split each expert into tensor_scalar
                # (bf16 4x mode) + bf16 tensor_tensor add (2x mode).
                cb = combpool.tile([128, EBLK], dt.bfloat16, tag="cb")
                combs[s] = cb
                rb = rbs[s]
                for oh in range(2):
                    dstc = cb[:, oh * OHBLK : (oh + 1) * OHBLK]
                    for e in range(E):
                        src = w_sb[:, e * EBLK + oh * OHBLK : e * EBLK + (oh + 1) * OHBLK]
                        if e == 0:
                            nc.vector.tensor_scalar_mul(dstc, src, rb[:, 0:1])
                        else:
                            tmp = spool.tile([128, OHBLK], dt.bfloat16, tag="stmp")
                            nc.vector.tensor_scalar_mul(tmp[:], src, rb[:, e : e + 1])
                            nc.vector.tensor_tensor(
                                out=dstc, in0=tmp[:], in1=dstc, op=Alu.add
                            )

            def conv(s):
                xv = xvs[s]
                cb = combs[s]
                for oh in range(2):
                    for nt in range(NT):
                        r0 = nt * ROWS
                        ps = pspool.tile([128, NF], dt.float32, tag="ps")
                        k = 0
                        for ci in range(2):
                            for kh in range(3):
                                for kw in range(3):
                                    tap = kh * 3 + kw
                                    lo = oh * OHBLK + ci * CIBLK + tap * OIN
                                    nc.tensor.matmul(
                                        ps[:],
                                        lhsT=cb[:, lo : lo + OIN],
                                        rhs=xv[
                                            :,
                                            ci,
                                            r0 + kh : r0 + kh + ROWS,
                                            kw + 1 : kw + 1 + W,
                                        ],
                                        start=(k == 0),
                                        stop=(k == 17),
                                    )
                                    k += 1
                        ot = opool.tile([128, NF], dt.float32, tag="ot")
                        nc.scalar.activation(out=ot[:], in_=ps[:], func=AF.Copy)
                        nc.scalar.dma_start(
                            out=y_d.ap()[s, oh * 128 : (oh + 1) * 128, r0 : r0 + ROWS, :],
                            in_=ot[:].rearrange("p (r w) -> p r w", r=ROWS, w=W),
                        )

            # ---- software-pipelined emission ----
            # All staging + routing chains first (they use DMA/ACT/GpSimd
            # plus tiny PE matmuls), then synthesis (DVE) interleaved with
            # the conv blocks (PE). Sample s+1's synthesis runs on DVE
            # while sample s's conv occupies PE.
            load_consts()
            stage(0, first=True)
            load_w()
            route(0)
            stage(1)
            route(1)
            stage(2)
            route(2)
            stage(3)
            route(3)
            synth(0)
            synth(1)
            conv(0)
            synth(2)
            conv(1)
            synth(3)
            conv(2)
            conv(3)

    nc.compile()
    return nc


def _get_nc():
    if "nc" not in _CACHE:
        _CACHE["nc"] = _build()
    return _CACHE["nc"]


def _pack_inputs(x, kernel_weights, fc_w, fc_b):
    # w layout per partition p (= i % 128): [e, oh, ci, tap, oin], bf16
    a = np.asarray(kernel_weights, np.float32).reshape(E, 2, 128, 2, 128, 3, 3)
    # dims: e, oh, oin, ci, p, kh, kw -> p, e, oh, ci, kh, kw, oin
    a = np.ascontiguousarray(a.transpose(4, 0, 1, 3, 5, 6, 2)).reshape(128, E * EBLK)
    wp = a.astype(ml_dtypes.bfloat16)
    fcw_t = np.ascontiguousarray(np.asarray(fc_w, np.float32).T / float(H * W))
    fcb2 = np.ascontiguousarray(np.asarray(fc_b, np.float32).reshape(E, 1))
    eye = np.eye(E, dtype=np.float32)
    x = np.ascontiguousarray(np.asarray(x, np.float32))
    in_maps = []
    for i in range(N_CORES):
        in_maps.append(
            {
                "x": x[i * BL : (i + 1) * BL],
                "wp": wp,
                "fcw": fcw_t,
                "fcb": fcb2,
                "eye": eye,
            }
        )
    return in_maps


def _run(x, kernel_weights, fc_w, fc_b, trace=False):
    from concourse.bass_utils import run_bass_kernel_spmd

    nc = _get_nc()
    in_maps = _pack_inputs(x, kernel_weights, fc_w, fc_b)
    res = run_bass_kernel_spmd(nc, in_maps, core_ids=list(range(N_CORES)), trace=trace)
    y = np.concatenate([res.results[i]["y"] for i in range(N_CORES)], axis=0)
    return np.ascontiguousarray(y.astype(np.float32)), res


def kernel(x, kernel_weights, fc_w, fc_b):
    y, _ = _run(x, kernel_weights, fc_w, fc_b, trace=False)
    return y


def kernel_traced(x, kernel_weights, fc_w, fc_b):
    y, res = _run(x, kernel_weights, fc_w, fc_b, trace=True)
    return y, res

